# revision 23
# baseline (speedup 1.0000x reference)
"""TransformerConv GNN message passing on 8 TRN2 NeuronCores (Bass/Tile).

Strategy (graph/edge parallelism, dst-sharded - no collectives needed):
  - Core c owns destination nodes [c*6250, (c+1)*6250); edges are sharded by
    their dst node, so the segment-softmax and scatter-aggregation are fully
    core-local.
  - Per the sharding hint, edges ship with their GATHERED node features:
    the host packs x[src], edge_attr (paired fp8 K-tiles for a DoubleRow
    matmul) and x[dst] (fp8) per 128-edge sub-chunk into one fused stream,
    plus a tiny f32 stream of window-local dst indices (one column per
    sub-chunk; -1 for padding edges).
  - On device, per dst-window of 128 nodes, per group of <=4 sub-chunks:
      kve = DoubleRow fp8 matmul: xs@[Wk|Wv] + ea@[We|We]  (one instr/chunk)
      qd  = xd@Wq (fp8)                                    (PE)
      oh  = is_equal(iota_row, dst_col)                    (DVE, replaces the
                                                            shipped onehot)
      alpha = rowsum_per_head(qd_sb * kve.k)               (DVE, bf16 out)
      pe  = exp(alpha/8)                                   (ACT, max-shift
                                                            dropped: identical)
      ve  = kve.v * pe_broadcast                           (Pool/GpSimd)
      [ve | pe] scatter: agg[128,130] += oh.T @ ve         (PE)
    Window epilogue: out = (agg/denom) @ Wproj + x_own @ (Wskip@Wproj) + bias.
  - Softmax normalization is applied after aggregation (linearity); padding
    edges carry dst=-1 so their generated onehot row is all-zero.

kernel(**inputs) takes the FULL unsharded inputs and returns the FULL
[50000, 128] float32 output.  Set TRACE=True to capture NTFF timing
(LAST_EXEC_TIME_NS / LAST_RESULTS are populated).
"""
import sys
from contextlib import ExitStack

import numpy as np

for _p in ('/opt/trn_rl_repo', '/root/.axon_site/_ro/trn_rl_repo'):
    if _p not in sys.path:
        sys.path.append(_p)

import ml_dtypes

import concourse.bass as bass          # noqa: E402
import concourse.mybir as mybir        # noqa: E402
import concourse.tile as tile          # noqa: E402
from concourse import bacc             # noqa: E402
from concourse import bass_utils       # noqa: E402

bf16 = ml_dtypes.bfloat16
f8 = ml_dtypes.float8_e4m3
F32 = mybir.dt.float32
BF16 = mybir.dt.bfloat16
FP8 = mybir.dt.float8e4

N = 50000
E = 800000
DIM = 128
H = 2
C = 64
P = 128
NCORES = 8
NODES_PER_CORE = N // NCORES          # 6250
WIN = 128
NWIN = (NODES_PER_CORE + WIN - 1) // WIN   # 49
NODES_PAD = NWIN * WIN                # 6272
GROUP = 4
ALPHA_SCALE = 0.125                   # 1/sqrt(64)
SUBW = 5 * P                          # fp8-byte cols per sub-chunk: xs|ea|oh|xd(bf16 as 2B)

TRACE = False
LAST_EXEC_TIME_NS = None
LAST_RESULTS = None


# ----------------------------------------------------------------------------
# host-side sharding / preprocessing
# ----------------------------------------------------------------------------

def _schedule(S):
    groups = []
    sub_base = 0
    for w in range(NWIN):
        for g0 in range(0, S[w], GROUP):
            Wg = min(GROUP, S[w] - g0)
            groups.append((w, sub_base + g0, Wg))
        sub_base += S[w]
    return groups, sub_base


def _prep(x, edge_attr, edge_index):
    x_np = np.asarray(x, dtype=np.float32)
    src = np.asarray(edge_index[0], dtype=np.int64)
    dst = np.asarray(edge_index[1], dtype=np.int64)

    core_of = dst // NODES_PER_CORE
    dst_local = dst - core_of * NODES_PER_CORE
    win_of = dst_local // WIN

    counts = np.zeros((NCORES, NWIN), dtype=np.int64)
    np.add.at(counts, (core_of, win_of), 1)
    S = np.maximum(np.ceil(counts / P).astype(np.int64).max(axis=0), 1)
    TS = int(S.sum())
    EPAD = TS * P

    order = np.lexsort((np.arange(E), win_of, core_of))
    run_ends = np.cumsum(counts.reshape(-1))
    run_starts = np.concatenate([[0], run_ends[:-1]]).reshape(NCORES, NWIN)
    run_ends = run_ends.reshape(NCORES, NWIN)

    ea_np = np.asarray(edge_attr, dtype=np.float32)
    wbase = np.concatenate([[0], np.cumsum(S)])
    per_core = []
    for c in range(NCORES):
        src_pad = np.zeros(EPAD, dtype=np.int64)
        dstg_pad = np.zeros(EPAD, dtype=np.int64)
        dstoh_pad = np.full(EPAD, -1, dtype=np.int64)
        ea_rows = np.zeros(EPAD, dtype=np.int64)
        ea_valid = np.zeros(EPAD, dtype=bool)
        for w in range(NWIN):
            sel = order[run_starts[c, w]:run_ends[c, w]]
            cnt = len(sel)
            base = int(wbase[w]) * P
            src_pad[base:base + cnt] = src[sel]
            dstg_pad[base:base + cnt] = dst[sel]
            dstoh_pad[base:base + cnt] = dst_local[sel] - w * WIN
            ea_rows[base:base + cnt] = sel
            ea_valid[base:base + cnt] = True

        ea = np.zeros((EPAD, DIM), dtype=np.float32)
        ea[ea_valid] = ea_np[ea_rows[ea_valid]]
        xs = x_np[src_pad]
        xs[~ea_valid] = 0.0
        xd = x_np[dstg_pad]
        oh = np.zeros((EPAD, P), dtype=np.float32)   # [edges, window-nodes]
        vmask = dstoh_pad >= 0
        oh[np.nonzero(vmask)[0], dstoh_pad[vmask]] = 1.0

        def sub_t(mat):   # feature dim on partitions, per 128-edge sub-chunk
            return np.ascontiguousarray(
                mat.reshape(TS, P, P).transpose(2, 0, 1)).reshape(P, EPAD)

        def sub_n(mat):   # edges on partitions (onehot)
            return np.ascontiguousarray(
                mat.reshape(TS, P, P).transpose(1, 0, 2)).reshape(P, EPAD)

        xsT = sub_t(xs).astype(f8)
        eaT = sub_t(ea).astype(f8)
        ohN = sub_n(oh).astype(f8)
        xdT = sub_t(xd).astype(bf16)
        xdB = np.ascontiguousarray(xdT).view(np.uint8)   # [P, TS*P*2] raw bytes

        edge8 = np.empty((P, TS * SUBW), dtype=f8)
        e8u = edge8.view(np.uint8)
        for s in range(TS):
            o = s * SUBW
            edge8[:, o:o + P] = xsT[:, s * P:(s + 1) * P]
            edge8[:, o + P:o + 2 * P] = eaT[:, s * P:(s + 1) * P]
            edge8[:, o + 2 * P:o + 3 * P] = ohN[:, s * P:(s + 1) * P]
            e8u[:, o + 3 * P:o + 5 * P] = xdB[:, s * 2 * P:(s + 1) * 2 * P]

        per_core.append(edge8)

    return per_core, dict(S=S.tolist(), TS=TS)


def _device_inputs(inputs):
    x = np.asarray(inputs['x'], dtype=np.float32)
    per_core, sched = _prep(x, inputs['edge_attr'], inputs['edge_index'])
    ident = np.eye(P, dtype=np.float32).astype(bf16)
    biases = {k: np.asarray(inputs[k], dtype=np.float32)
              for k in ['bq', 'bk', 'bv', 'bskip', 'bproj']}
    has_bias = any(np.any(b != 0) for b in biases.values())

    wk = np.asarray(inputs['Wk'], dtype=np.float32)
    wv = np.asarray(inputs['Wv'], dtype=np.float32)
    we = np.asarray(inputs['We'], dtype=np.float32)
    w2f = np.empty((P, 2, 2 * P), dtype=np.float32)  # K-tile 0: [Wk|Wv], 1: [We|We]
    w2f[:, 0, 0:P] = wk
    w2f[:, 0, P:2 * P] = wv
    w2f[:, 1, 0:P] = we
    w2f[:, 1, P:2 * P] = we
    w2 = w2f.astype(f8)
    w2res = (w2f - w2.astype(np.float32)).astype(f8)  # fp8 quantization residual

    in_maps = []
    for c in range(NCORES):
        own = np.zeros((NODES_PAD, DIM), dtype=np.float32)
        own[:NODES_PER_CORE] = x[c * NODES_PER_CORE:(c + 1) * NODES_PER_CORE]
        m = dict(
            edge8=per_core[c],
            xTown_pm=np.ascontiguousarray(own.T).astype(bf16),
            ident_in=ident,
            w2_pm=w2,
            w2res_pm=w2res,
            wq=np.asarray(inputs['Wq'], dtype=np.float32),
            wskip=np.asarray(inputs['Wskip'], dtype=np.float32),
            wproj=np.asarray(inputs['Wproj'], dtype=np.float32),
        )
        if has_bias:
            m['bkv_row'] = np.ascontiguousarray(
                np.concatenate([biases['bk'], biases['bv']])[None, :])
            m['bq_row'] = np.ascontiguousarray(biases['bq'][None, :])
            m['bskip_col'] = np.ascontiguousarray(biases['bskip'][:, None])
            m['bproj_row'] = np.ascontiguousarray(biases['bproj'][None, :])
        in_maps.append(m)
    return sched, in_maps, has_bias


# ----------------------------------------------------------------------------
# device kernel
# ----------------------------------------------------------------------------

def _build(sched, has_bias=False):
    S = sched['S']
    groups, TS = _schedule(S)
    nc = bacc.Bacc("TRN2", target_bir_lowering=False, debug=False)

    edge8 = nc.dram_tensor("edge8", [P, TS * SUBW], FP8, kind="ExternalInput").ap()
    xTown_pm = nc.dram_tensor("xTown_pm", [P, NODES_PAD], BF16, kind="ExternalInput").ap()
    ident_in = nc.dram_tensor("ident_in", [P, P], BF16, kind="ExternalInput").ap()
    w2_pm = nc.dram_tensor("w2_pm", [P, 2, 2 * P], FP8, kind="ExternalInput").ap()
    w2res_pm = nc.dram_tensor("w2res_pm", [P, 2, 2 * P], FP8, kind="ExternalInput").ap()
    w_in = {}
    for name in ["wq", "wskip", "wproj"]:
        w_in[name] = nc.dram_tensor(name, [P, P], F32, kind="ExternalInput").ap()
    if has_bias:
        bkv_row = nc.dram_tensor("bkv_row", [1, 2 * P], F32, kind="ExternalInput").ap()
        bq_row = nc.dram_tensor("bq_row", [1, P], F32, kind="ExternalInput").ap()
        bskip_col = nc.dram_tensor("bskip_col", [P, 1], F32, kind="ExternalInput").ap()
        bproj_row = nc.dram_tensor("bproj_row", [1, P], F32, kind="ExternalInput").ap()
    out = nc.dram_tensor("out", [NODES_PAD, DIM], F32, kind="ExternalOutput").ap()

    with tile.TileContext(nc) as tc, ExitStack() as top:
        res = top.enter_context(tc.tile_pool(name="res", bufs=1))

        xTown_sb = res.tile([P, NODES_PAD], BF16)
        nc.sync.dma_start(out=xTown_sb[:], in_=xTown_pm[:, :])
        ident = res.tile([P, P], BF16)
        nc.sync.dma_start(out=ident[:], in_=ident_in[:, :])
        w2_sb = res.tile([P, 2, 2 * P], FP8)
        nc.sync.dma_start(out=w2_sb[:], in_=w2_pm[:, :, :])
        w2res_sb = res.tile([P, 2, 2 * P], FP8)
        nc.sync.dma_start(out=w2res_sb[:], in_=w2res_pm[:, :, :])

        wsb = {}
        for name in ["wq", "wskip", "wproj"]:
            wf = res.tile([P, P], F32, tag="wf32")
            nc.sync.dma_start(out=wf[:], in_=w_in[name][:, :])
            wb = res.tile([P, P], BF16, tag=f"{name}_b")
            nc.vector.tensor_copy(out=wb[:], in_=wf[:])
            wsb[name] = wb

        if has_bias:
            bkv_sb = res.tile([1, 2 * P], BF16)
            bq_sb = res.tile([1, P], BF16)
            ones_row = res.tile([1, P], BF16)
            nc.vector.memset(ones_row[:], 1.0)
            bkvf = res.tile([1, 2 * P], F32)
            nc.sync.dma_start(out=bkvf[:], in_=bkv_row[:, :])
            nc.vector.tensor_copy(out=bkv_sb[:], in_=bkvf[:])
            bqf = res.tile([1, P], F32)
            nc.sync.dma_start(out=bqf[:], in_=bq_row[:, :])
            nc.vector.tensor_copy(out=bq_sb[:], in_=bqf[:])
            bskipc = res.tile([P, 1], F32)
            nc.sync.dma_start(out=bskipc[:], in_=bskip_col[:, :])
            bskipc_b = res.tile([P, 1], BF16)
            nc.vector.tensor_copy(out=bskipc_b[:], in_=bskipc[:])
            bprojf = res.tile([1, P], F32)
            nc.sync.dma_start(out=bprojf[:], in_=bproj_row[:, :])

        # fused skip weight: Wfused = Wskip @ Wproj  (and fused bias)
        wfused_sb = res.tile([P, P], BF16)
        bfused_sb = res.tile([1, P], BF16, name="bfused_sb") if has_bias else None
        with tc.tile_pool(name="wset_ps", bufs=1, space="PSUM") as wps_pool, \
             tc.tile_pool(name="wset_sb", bufs=1) as wsb_pool:
            tp = wps_pool.tile([P, P], BF16)
            nc.tensor.transpose(out=tp[:], in_=wsb["wskip"][:], identity=ident[:])
            wskipT = wsb_pool.tile([P, P], BF16)
            nc.vector.tensor_copy(out=wskipT[:], in_=tp[:])
            wf_ps = wps_pool.tile([P, P], F32)
            nc.tensor.matmul(out=wf_ps[:], lhsT=wskipT[:], rhs=wsb["wproj"][:],
                             start=True, stop=True)
            nc.vector.tensor_copy(out=wfused_sb[:], in_=wf_ps[:])
            if has_bias:
                bf_ps = wps_pool.tile([1, P], F32)
                nc.tensor.matmul(out=bf_ps[:], lhsT=bskipc_b[:], rhs=wsb["wproj"][:],
                                 start=True, stop=True)
                bff = wsb_pool.tile([1, P], F32)
                nc.vector.tensor_add(out=bff[:], in0=bf_ps[:], in1=bprojf[:])
                nc.vector.tensor_copy(out=bfused_sb[:], in_=bff[:])

        # ---------------- main loop (3-stage software pipeline) -------------
        with tc.tile_pool(name="edge_in", bufs=8) as in_pool, \
             tc.tile_pool(name="work", bufs=10) as wk_pool, \
             tc.tile_pool(name="kve_ps", bufs=3, space="PSUM") as kve_pool, \
             tc.tile_pool(name="qd_ps", bufs=1, space="PSUM") as qd_pool, \
             tc.tile_pool(name="agg_ps", bufs=1, space="PSUM") as agg_pool, \
             tc.tile_pool(name="outp", bufs=8) as out_pool:
            aggs = {}

            def epilogue(w):
                agg = aggs.pop(w)
                den = out_pool.tile([P, H], F32, tag="den", name=f"den{w}")
                nc.vector.tensor_scalar_add(den[:], agg[:, P:P + H], 1e-30)
                inv = out_pool.tile([P, H], F32, tag="inv", name=f"inv{w}")
                nc.vector.reciprocal(out=inv[:], in_=den[:])
                aggn = out_pool.tile([P, P], BF16, tag="aggn", name=f"aggn{w}")
                nc.vector.tensor_mul(
                    out=aggn[:].rearrange("p (h c) -> p h c", c=C),
                    in0=agg[:, 0:P].rearrange("p (h c) -> p h c", c=C),
                    in1=inv[:].unsqueeze(2).broadcast_to([P, H, C]))
                tp_ps = agg_pool.tile([P, P], BF16, tag="agg", name=f"tp{w}")
                nc.tensor.transpose(out=tp_ps[:], in_=aggn[:], identity=ident[:])
                aggT = out_pool.tile([P, P], BF16, tag="aggT", name=f"aggT{w}")
                nc.scalar.copy(out=aggT[:], in_=tp_ps[:])
                fin = agg_pool.tile([P, P], F32, tag="agg", name=f"fin{w}")
                nc.tensor.matmul(out=fin[:], lhsT=aggT[:], rhs=wsb["wproj"][:],
                                 start=True, stop=False, skip_group_check=True)
                nc.tensor.matmul(out=fin[:], lhsT=xTown_sb[:, w * P:(w + 1) * P],
                                 rhs=wfused_sb[:], start=False,
                                 stop=not has_bias, skip_group_check=True)
                if has_bias:
                    nc.tensor.matmul(out=fin[:], lhsT=ones_row[:], rhs=bfused_sb[:],
                                     start=False, stop=True, skip_group_check=True)
                fin_sb = out_pool.tile([P, P], F32, tag="fin_sb", name=f"fsb{w}")
                nc.scalar.copy(out=fin_sb[:], in_=fin[:])
                nc.sync.dma_start(out=out[w * P:(w + 1) * P, :], in_=fin_sb[:])

            def stage_C(st):
                Wg = st['Wg']
                qk = wk_pool.tile([P, Wg, P], BF16, tag="qk", name=f"qk{st['s0']}")
                nc.vector.tensor_mul(out=qk[:], in0=st['qd_sb'][:],
                                     in1=st['kve'][:, 0:Wg, 0:P])
                alpha = wk_pool.tile([P, Wg, H], F32, tag="alpha",
                                     name=f"al{st['s0']}")
                nc.vector.reduce_sum(
                    out=alpha[:],
                    in_=qk[:].rearrange("p j (h c) -> p (j h) c", c=C),
                    axis=mybir.AxisListType.X)
                pe_x = wk_pool.tile([P, Wg, P], BF16, tag="pe_x",
                                    name=f"pe{st['s0']}")
                nc.scalar.activation(
                    out=pe_x[:],
                    in_=alpha[:].unsqueeze(3).broadcast_to([P, Wg, H, C]),
                    func=mybir.ActivationFunctionType.Exp, scale=ALPHA_SCALE)
                st['alpha'] = alpha
                st['pe_x'] = pe_x

            def stage_D(st):
                Wg = st['Wg']
                w, s0 = st['w'], st['s0']
                ve = wk_pool.tile([P, Wg, 130], BF16, tag="ve", name=f"ve{s0}")
                nc.vector.tensor_mul(
                    out=ve[:, :, 0:P],
                    in0=st['kve'][:, 0:Wg, P:2 * P],
                    in1=st['pe_x'][:])
                nc.scalar.activation(
                    out=ve[:, :, P:P + H], in_=st['alpha'][:],
                    func=mybir.ActivationFunctionType.Exp, scale=ALPHA_SCALE)
                Sw = S[w]
                wstart = sum(S[:w])
                blk = st['blk']
                for j in range(Wg):
                    nd = s0 - wstart + j
                    nc.tensor.matmul(
                        out=aggs[w][:],
                        lhsT=blk[:, j * SUBW + 2 * P:j * SUBW + 3 * P],
                        rhs=ve[:, j, :],
                        start=(nd == 0), stop=(nd == Sw - 1),
                        skip_group_check=True)
                if s0 - wstart + Wg == Sw:
                    epilogue(w)

            stC = None
            stD = None
            cur_w = -1
            for (w, s0, Wg) in groups:
                if w != cur_w:
                    cur_w = w
                    aggs[w] = agg_pool.tile([P, 130], F32, tag="agg", name=f"agg{w}")

                blk = in_pool.tile([P, Wg * SUBW], FP8, tag="blk")
                nc.sync.dma_start(out=blk[:],
                                  in_=edge8[:, s0 * SUBW:(s0 + Wg) * SUBW])

                kve = kve_pool.tile([P, GROUP, 2 * P], F32, tag="kve")
                qd = qd_pool.tile([P, GROUP, P], F32, tag="qd")
                for j in range(Wg):
                    kvp = blk[:, j * SUBW:j * SUBW + 2 * P].rearrange(
                        "p (two m) -> p two m", two=2)
                    nc.tensor.matmul(
                        out=kve[:, j, :], lhsT=kvp, rhs=w2_sb[:],
                        start=True, stop=False,
                        perf_mode=mybir.MatmulPerfMode.DoubleRow,
                        skip_group_check=True)
                    nc.tensor.matmul(
                        out=kve[:, j, :], lhsT=kvp, rhs=w2res_sb[:],
                        start=False, stop=not has_bias,
                        perf_mode=mybir.MatmulPerfMode.DoubleRow,
                        skip_group_check=True)
                    if has_bias:
                        nc.tensor.matmul(out=kve[:, j, :], lhsT=ones_row[:],
                                         rhs=bkv_sb[:], start=False, stop=True,
                                         skip_group_check=True)
                for j in range(Wg):
                    xd_j = blk[:, j * SUBW + 3 * P:(j + 1) * SUBW].bitcast(BF16)
                    nc.tensor.matmul(out=qd[:, j, :],
                                     lhsT=xd_j,
                                     rhs=wsb["wq"][:], start=True,
                                     stop=not has_bias, skip_group_check=True)
                    if has_bias:
                        nc.tensor.matmul(out=qd[:, j, :], lhsT=ones_row[:],
                                         rhs=bq_sb[:], start=False, stop=True,
                                         skip_group_check=True)

                qd_sb = wk_pool.tile([P, Wg, P], BF16, tag="qd_sb")
                nc.scalar.copy(out=qd_sb[:], in_=qd[:, 0:Wg, :])

                if stC is not None:
                    stage_C(stC)
                if stD is not None:
                    stage_D(stD)

                stD = stC
                stC = dict(w=w, s0=s0, Wg=Wg, kve=kve, qd_sb=qd_sb, blk=blk)

            stage_C(stC)
            stage_D(stD)
            stage_D(stC)

    nc.compile()
    return nc


# ----------------------------------------------------------------------------
# entry point
# ----------------------------------------------------------------------------

def kernel(**inputs):
    global LAST_EXEC_TIME_NS, LAST_RESULTS
    assert np.asarray(inputs['x']).shape == (N, DIM)
    assert np.asarray(inputs['edge_index']).shape == (2, E)

    sched, in_maps, has_bias = _device_inputs(inputs)
    nc = _build(sched, has_bias=has_bias)
    res = bass_utils.run_bass_kernel_spmd(
        nc, in_maps, core_ids=list(range(NCORES)), trace=TRACE)
    LAST_EXEC_TIME_NS = res.exec_time_ns
    LAST_RESULTS = res
    outs = [r['out'][:NODES_PER_CORE] for r in res.results]
    return np.ascontiguousarray(
        np.concatenate(outs, axis=0).astype(np.float32))


# revision 37
# speedup vs baseline: 1.0349x; 1.0349x over previous
"""TransformerConv GNN message passing on 8 TRN2 NeuronCores (Bass/Tile).

Strategy (graph/edge parallelism, dst-sharded - no collectives needed):
  - Core c owns destination nodes [c*6250, (c+1)*6250); edges are sharded by
    their dst node, so the segment-softmax and scatter-aggregation are fully
    core-local.
  - Per the sharding hint, edges ship with their GATHERED node features:
    the host packs x[src], edge_attr (paired fp8 K-tiles for a DoubleRow
    matmul) and x[dst] (fp8) per 128-edge sub-chunk into one fused stream,
    plus a tiny f32 stream of window-local dst indices (one column per
    sub-chunk; -1 for padding edges).
  - On device, per dst-window of 128 nodes, per group of <=4 sub-chunks:
      kve = DoubleRow fp8 matmul: xs@[Wk|Wv] + ea@[We|We]  (one instr/chunk)
      qd  = xd@Wq (fp8)                                    (PE)
      oh  = is_equal(iota_row, dst_col)                    (DVE, replaces the
                                                            shipped onehot)
      alpha = rowsum_per_head(qd_sb * kve.k)               (DVE, bf16 out)
      pe  = exp(alpha/8)                                   (ACT, max-shift
                                                            dropped: identical)
      ve  = kve.v * pe_broadcast                           (Pool/GpSimd)
      [ve | pe] scatter: agg[128,130] += oh.T @ ve         (PE)
    Window epilogue: out = (agg/denom) @ Wproj + x_own @ (Wskip@Wproj) + bias.
  - Softmax normalization is applied after aggregation (linearity); padding
    edges carry dst=-1 so their generated onehot row is all-zero.

kernel(**inputs) takes the FULL unsharded inputs and returns the FULL
[50000, 128] float32 output.  Set TRACE=True to capture NTFF timing
(LAST_EXEC_TIME_NS / LAST_RESULTS are populated).
"""
import sys
from contextlib import ExitStack

import numpy as np

for _p in ('/opt/trn_rl_repo', '/root/.axon_site/_ro/trn_rl_repo'):
    if _p not in sys.path:
        sys.path.append(_p)

import ml_dtypes

import concourse.bass as bass          # noqa: E402
import concourse.mybir as mybir        # noqa: E402
import concourse.tile as tile          # noqa: E402
from concourse import bacc             # noqa: E402
from concourse import bass_utils       # noqa: E402

bf16 = ml_dtypes.bfloat16
f8 = ml_dtypes.float8_e4m3
F32 = mybir.dt.float32
BF16 = mybir.dt.bfloat16
FP8 = mybir.dt.float8e4

N = 50000
E = 800000
DIM = 128
H = 2
C = 64
P = 128
NCORES = 8
NODES_PER_CORE = N // NCORES          # 6250
WIN = 128
NWIN = (NODES_PER_CORE + WIN - 1) // WIN   # 49
NODES_PAD = NWIN * WIN                # 6272
GROUP = 4
ALPHA_SCALE = 0.125                   # 1/sqrt(64)
SUBW = 5 * P                          # fp8-byte cols per sub-chunk: xs|ea|oh|xd(bf16 as 2B)

TRACE = False
LAST_EXEC_TIME_NS = None
LAST_RESULTS = None


# ----------------------------------------------------------------------------
# host-side sharding / preprocessing
# ----------------------------------------------------------------------------

def _schedule(S):
    groups = []
    sub_base = 0
    for w in range(NWIN):
        for g0 in range(0, S[w], GROUP):
            Wg = min(GROUP, S[w] - g0)
            groups.append((w, sub_base + g0, Wg))
        sub_base += S[w]
    return groups, sub_base


def _balance(dst):
    """Greedy balanced assignment of nodes to (core, window) bins.

    Returns (bin_of[N], slot_of[N]): bin b holds exactly WIN nodes; node n sits
    at window-local slot slot_of[n].  Bins are load-balanced by in-degree so
    every bin has ~E/(NCORES*NWIN) incoming edges, minimizing sub-chunk padding.
    """
    import heapq
    deg = np.bincount(dst, minlength=N)
    NB = NCORES * NWIN
    order = np.argsort(-deg, kind='stable')
    heap = [(0, b) for b in range(NB)]
    heapq.heapify(heap)
    slots = np.zeros(NB, np.int32)
    bin_of = np.empty(N, np.int32)
    slot_of = np.empty(N, np.int32)
    for n in order:
        while True:
            load, b = heapq.heappop(heap)
            if slots[b] < WIN:
                break
        bin_of[n] = b
        slot_of[n] = slots[b]
        slots[b] += 1
        if slots[b] < WIN:
            heapq.heappush(heap, (load + int(deg[n]), b))
    return bin_of, slot_of


def _prep(x, edge_attr, edge_index):
    x_np = np.asarray(x, dtype=np.float32)
    src = np.asarray(edge_index[0], dtype=np.int64)
    dst = np.asarray(edge_index[1], dtype=np.int64)

    bin_of, slot_of = _balance(dst)
    core_of = (bin_of // NWIN)[dst]
    win_of = (bin_of % NWIN)[dst]
    dst_slot = slot_of[dst]

    counts = np.zeros((NCORES, NWIN), dtype=np.int64)
    np.add.at(counts, (core_of, win_of), 1)
    S = np.maximum(np.ceil(counts / P).astype(np.int64).max(axis=0), 1)
    TS = int(S.sum())
    EPAD = TS * P

    order = np.lexsort((np.arange(E), win_of, core_of))
    run_ends = np.cumsum(counts.reshape(-1))
    run_starts = np.concatenate([[0], run_ends[:-1]]).reshape(NCORES, NWIN)
    run_ends = run_ends.reshape(NCORES, NWIN)

    ea_np = np.asarray(edge_attr, dtype=np.float32)
    wbase = np.concatenate([[0], np.cumsum(S)])
    per_core = []
    for c in range(NCORES):
        src_pad = np.zeros(EPAD, dtype=np.int64)
        dstg_pad = np.zeros(EPAD, dtype=np.int64)
        dstoh_pad = np.full(EPAD, -1, dtype=np.int64)
        ea_rows = np.zeros(EPAD, dtype=np.int64)
        ea_valid = np.zeros(EPAD, dtype=bool)
        for w in range(NWIN):
            sel = order[run_starts[c, w]:run_ends[c, w]]
            cnt = len(sel)
            base = int(wbase[w]) * P
            src_pad[base:base + cnt] = src[sel]
            dstg_pad[base:base + cnt] = dst[sel]
            dstoh_pad[base:base + cnt] = dst_slot[sel]
            ea_rows[base:base + cnt] = sel
            ea_valid[base:base + cnt] = True

        ea = np.zeros((EPAD, DIM), dtype=np.float32)
        ea[ea_valid] = ea_np[ea_rows[ea_valid]]
        xs = x_np[src_pad]
        xs[~ea_valid] = 0.0
        xd = x_np[dstg_pad]
        oh = np.zeros((EPAD, P), dtype=np.float32)   # [edges, window-nodes]
        vmask = dstoh_pad >= 0
        oh[np.nonzero(vmask)[0], dstoh_pad[vmask]] = 1.0

        def sub_t(mat):   # feature dim on partitions, per 128-edge sub-chunk
            return np.ascontiguousarray(
                mat.reshape(TS, P, P).transpose(2, 0, 1)).reshape(P, EPAD)

        def sub_n(mat):   # edges on partitions (onehot)
            return np.ascontiguousarray(
                mat.reshape(TS, P, P).transpose(1, 0, 2)).reshape(P, EPAD)

        xsT = sub_t(xs).astype(f8)
        eaT = sub_t(ea).astype(f8)
        ohN = sub_n(oh).astype(f8)
        xdT = sub_t(xd).astype(bf16)
        xdB = np.ascontiguousarray(xdT).view(np.uint8)   # [P, TS*P*2] raw bytes

        edge8 = np.empty((P, TS * SUBW), dtype=f8)
        e8u = edge8.view(np.uint8)
        for s in range(TS):
            o = s * SUBW
            edge8[:, o:o + P] = xsT[:, s * P:(s + 1) * P]
            edge8[:, o + P:o + 2 * P] = eaT[:, s * P:(s + 1) * P]
            edge8[:, o + 2 * P:o + 3 * P] = ohN[:, s * P:(s + 1) * P]
            e8u[:, o + 3 * P:o + 5 * P] = xdB[:, s * 2 * P:(s + 1) * 2 * P]

        per_core.append(edge8)

    # global output row of each node: core*NODES_PAD + window*P + slot
    node_row = ((bin_of // NWIN).astype(np.int64) * NODES_PAD
                + (bin_of % NWIN).astype(np.int64) * P
                + slot_of.astype(np.int64))
    return per_core, dict(S=S.tolist(), TS=TS), node_row


def _device_inputs(inputs):
    x = np.asarray(inputs['x'], dtype=np.float32)
    per_core, sched, node_row = _prep(x, inputs['edge_attr'],
                                      inputs['edge_index'])
    ident = np.eye(P, dtype=np.float32).astype(bf16)
    biases = {k: np.asarray(inputs[k], dtype=np.float32)
              for k in ['bq', 'bk', 'bv', 'bskip', 'bproj']}
    has_bias = any(np.any(b != 0) for b in biases.values())

    wk = np.asarray(inputs['Wk'], dtype=np.float32)
    wv = np.asarray(inputs['Wv'], dtype=np.float32)
    we = np.asarray(inputs['We'], dtype=np.float32)
    w2f = np.empty((P, 2, 2 * P), dtype=np.float32)  # K-tile 0: [Wk|Wv], 1: [We|We]
    w2f[:, 0, 0:P] = wk
    w2f[:, 0, P:2 * P] = wv
    w2f[:, 1, 0:P] = we
    w2f[:, 1, P:2 * P] = we
    w2 = w2f.astype(f8)

    # xTown rows follow the balanced (core, window, slot) node layout
    xext = np.zeros((NCORES * NODES_PAD, DIM), dtype=np.float32)
    xext[node_row] = x

    in_maps = []
    for c in range(NCORES):
        own = xext[c * NODES_PAD:(c + 1) * NODES_PAD]
        m = dict(
            edge8=per_core[c],
            xTown_pm=np.ascontiguousarray(own.T).astype(bf16),
            ident_in=ident,
            w2_pm=w2,
            wq=np.asarray(inputs['Wq'], dtype=np.float32),
            wskip=np.asarray(inputs['Wskip'], dtype=np.float32),
            wproj=np.asarray(inputs['Wproj'], dtype=np.float32),
        )
        if has_bias:
            m['bkv_row'] = np.ascontiguousarray(
                np.concatenate([biases['bk'], biases['bv']])[None, :])
            m['bq_row'] = np.ascontiguousarray(biases['bq'][None, :])
            m['bskip_col'] = np.ascontiguousarray(biases['bskip'][:, None])
            m['bproj_row'] = np.ascontiguousarray(biases['bproj'][None, :])
        in_maps.append(m)
    return sched, in_maps, has_bias, node_row


# ----------------------------------------------------------------------------
# device kernel
# ----------------------------------------------------------------------------

def _build(sched, has_bias=False):
    S = sched['S']
    groups, TS = _schedule(S)
    nc = bacc.Bacc("TRN2", target_bir_lowering=False, debug=False)

    edge8 = nc.dram_tensor("edge8", [P, TS * SUBW], FP8, kind="ExternalInput").ap()
    xTown_pm = nc.dram_tensor("xTown_pm", [P, NODES_PAD], BF16, kind="ExternalInput").ap()
    ident_in = nc.dram_tensor("ident_in", [P, P], BF16, kind="ExternalInput").ap()
    w2_pm = nc.dram_tensor("w2_pm", [P, 2, 2 * P], FP8, kind="ExternalInput").ap()
    w_in = {}
    for name in ["wq", "wskip", "wproj"]:
        w_in[name] = nc.dram_tensor(name, [P, P], F32, kind="ExternalInput").ap()
    if has_bias:
        bkv_row = nc.dram_tensor("bkv_row", [1, 2 * P], F32, kind="ExternalInput").ap()
        bq_row = nc.dram_tensor("bq_row", [1, P], F32, kind="ExternalInput").ap()
        bskip_col = nc.dram_tensor("bskip_col", [P, 1], F32, kind="ExternalInput").ap()
        bproj_row = nc.dram_tensor("bproj_row", [1, P], F32, kind="ExternalInput").ap()
    out = nc.dram_tensor("out", [NODES_PAD, DIM], F32, kind="ExternalOutput").ap()

    with tile.TileContext(nc) as tc, ExitStack() as top:
        res = top.enter_context(tc.tile_pool(name="res", bufs=1))

        xTown_sb = res.tile([P, NODES_PAD], BF16)
        nc.sync.dma_start(out=xTown_sb[:], in_=xTown_pm[:, :])
        ident = res.tile([P, P], BF16)
        nc.sync.dma_start(out=ident[:], in_=ident_in[:, :])
        w2_sb = res.tile([P, 2, 2 * P], FP8)
        nc.sync.dma_start(out=w2_sb[:], in_=w2_pm[:, :, :])

        wsb = {}
        for name in ["wq", "wskip", "wproj"]:
            wf = res.tile([P, P], F32, tag="wf32")
            nc.sync.dma_start(out=wf[:], in_=w_in[name][:, :])
            wb = res.tile([P, P], BF16, tag=f"{name}_b")
            nc.vector.tensor_copy(out=wb[:], in_=wf[:])
            wsb[name] = wb

        if has_bias:
            bkv_sb = res.tile([1, 2 * P], BF16)
            bq_sb = res.tile([1, P], BF16)
            ones_row = res.tile([1, P], BF16)
            nc.vector.memset(ones_row[:], 1.0)
            bkvf = res.tile([1, 2 * P], F32)
            nc.sync.dma_start(out=bkvf[:], in_=bkv_row[:, :])
            nc.vector.tensor_copy(out=bkv_sb[:], in_=bkvf[:])
            bqf = res.tile([1, P], F32)
            nc.sync.dma_start(out=bqf[:], in_=bq_row[:, :])
            nc.vector.tensor_copy(out=bq_sb[:], in_=bqf[:])
            bskipc = res.tile([P, 1], F32)
            nc.sync.dma_start(out=bskipc[:], in_=bskip_col[:, :])
            bskipc_b = res.tile([P, 1], BF16)
            nc.vector.tensor_copy(out=bskipc_b[:], in_=bskipc[:])
            bprojf = res.tile([1, P], F32)
            nc.sync.dma_start(out=bprojf[:], in_=bproj_row[:, :])

        # fused skip weight: Wfused = Wskip @ Wproj  (and fused bias)
        wfused_sb = res.tile([P, P], BF16)
        bfused_sb = res.tile([1, P], BF16, name="bfused_sb") if has_bias else None
        with tc.tile_pool(name="wset_ps", bufs=1, space="PSUM") as wps_pool, \
             tc.tile_pool(name="wset_sb", bufs=1) as wsb_pool:
            tp = wps_pool.tile([P, P], BF16)
            nc.tensor.transpose(out=tp[:], in_=wsb["wskip"][:], identity=ident[:])
            wskipT = wsb_pool.tile([P, P], BF16)
            nc.vector.tensor_copy(out=wskipT[:], in_=tp[:])
            wf_ps = wps_pool.tile([P, P], F32)
            nc.tensor.matmul(out=wf_ps[:], lhsT=wskipT[:], rhs=wsb["wproj"][:],
                             start=True, stop=True)
            nc.vector.tensor_copy(out=wfused_sb[:], in_=wf_ps[:])
            if has_bias:
                bf_ps = wps_pool.tile([1, P], F32)
                nc.tensor.matmul(out=bf_ps[:], lhsT=bskipc_b[:], rhs=wsb["wproj"][:],
                                 start=True, stop=True)
                bff = wsb_pool.tile([1, P], F32)
                nc.vector.tensor_add(out=bff[:], in0=bf_ps[:], in1=bprojf[:])
                nc.vector.tensor_copy(out=bfused_sb[:], in_=bff[:])

        # ---------------- main loop (3-stage software pipeline) -------------
        with tc.tile_pool(name="edge_in", bufs=12) as in_pool, \
             tc.tile_pool(name="work", bufs=10) as wk_pool, \
             tc.tile_pool(name="kve_ps", bufs=3, space="PSUM") as kve_pool, \
             tc.tile_pool(name="qd_ps", bufs=1, space="PSUM") as qd_pool, \
             tc.tile_pool(name="agg_ps", bufs=1, space="PSUM") as agg_pool, \
             tc.tile_pool(name="outp", bufs=8) as out_pool:
            aggs = {}

            def epilogue(w):
                agg = aggs.pop(w)
                den = out_pool.tile([P, H], F32, tag="den", name=f"den{w}")
                nc.vector.tensor_scalar_add(den[:], agg[:, P:P + H], 1e-30)
                inv = out_pool.tile([P, H], F32, tag="inv", name=f"inv{w}")
                nc.vector.reciprocal(out=inv[:], in_=den[:])
                aggn = out_pool.tile([P, P], BF16, tag="aggn", name=f"aggn{w}")
                nc.vector.tensor_mul(
                    out=aggn[:].rearrange("p (h c) -> p h c", c=C),
                    in0=agg[:, 0:P].rearrange("p (h c) -> p h c", c=C),
                    in1=inv[:].unsqueeze(2).broadcast_to([P, H, C]))
                tp_ps = agg_pool.tile([P, P], BF16, tag="agg", name=f"tp{w}")
                nc.tensor.transpose(out=tp_ps[:], in_=aggn[:], identity=ident[:])
                aggT = out_pool.tile([P, P], BF16, tag="aggT", name=f"aggT{w}")
                nc.scalar.copy(out=aggT[:], in_=tp_ps[:])
                fin = agg_pool.tile([P, P], F32, tag="agg", name=f"fin{w}")
                nc.tensor.matmul(out=fin[:], lhsT=aggT[:], rhs=wsb["wproj"][:],
                                 start=True, stop=False, skip_group_check=True)
                nc.tensor.matmul(out=fin[:], lhsT=xTown_sb[:, w * P:(w + 1) * P],
                                 rhs=wfused_sb[:], start=False,
                                 stop=not has_bias, skip_group_check=True)
                if has_bias:
                    nc.tensor.matmul(out=fin[:], lhsT=ones_row[:], rhs=bfused_sb[:],
                                     start=False, stop=True, skip_group_check=True)
                fin_sb = out_pool.tile([P, P], F32, tag="fin_sb", name=f"fsb{w}")
                nc.scalar.copy(out=fin_sb[:], in_=fin[:])
                nc.sync.dma_start(out=out[w * P:(w + 1) * P, :], in_=fin_sb[:])

            def stage_C(st):
                Wg = st['Wg']
                qk = wk_pool.tile([P, Wg, P], BF16, tag="qk", name=f"qk{st['s0']}")
                nc.vector.tensor_mul(out=qk[:], in0=st['qd_sb'][:],
                                     in1=st['kve'][:, 0:Wg, 0:P])
                alpha = wk_pool.tile([P, Wg, H], F32, tag="alpha",
                                     name=f"al{st['s0']}")
                nc.vector.reduce_sum(
                    out=alpha[:],
                    in_=qk[:].rearrange("p j (h c) -> p (j h) c", c=C),
                    axis=mybir.AxisListType.X)
                pe = wk_pool.tile([P, Wg, H], BF16, tag="pe", name=f"pe{st['s0']}")
                nc.scalar.activation(
                    out=pe[:], in_=alpha[:],
                    func=mybir.ActivationFunctionType.Exp, scale=ALPHA_SCALE)
                st['alpha'] = alpha
                st['pe'] = pe

            def stage_D(st):
                Wg = st['Wg']
                w, s0 = st['w'], st['s0']
                ve = wk_pool.tile([P, Wg, 130], BF16, tag="ve", name=f"ve{s0}")
                nc.vector.tensor_mul(
                    out=ve[:, :, 0:P].rearrange("p j (h c) -> p j h c", c=C),
                    in0=st['kve'][:, 0:Wg, P:2 * P].rearrange(
                        "p j (h c) -> p j h c", c=C),
                    in1=st['pe'][:].unsqueeze(3).broadcast_to([P, Wg, H, C]))
                nc.scalar.activation(
                    out=ve[:, :, P:P + H], in_=st['alpha'][:],
                    func=mybir.ActivationFunctionType.Exp, scale=ALPHA_SCALE)
                Sw = S[w]
                wstart = sum(S[:w])
                blk = st['blk']
                for j in range(Wg):
                    nd = s0 - wstart + j
                    nc.tensor.matmul(
                        out=aggs[w][:],
                        lhsT=blk[:, j * SUBW + 2 * P:j * SUBW + 3 * P],
                        rhs=ve[:, j, :],
                        start=(nd == 0), stop=(nd == Sw - 1),
                        skip_group_check=True)
                if s0 - wstart + Wg == Sw:
                    pending_epi.append(w)

            stC = None
            stD = None
            cur_w = -1
            pending_epi = []
            for (w, s0, Wg) in groups:
                if w != cur_w:
                    cur_w = w
                    aggs[w] = agg_pool.tile([P, 130], F32, tag="agg", name=f"agg{w}")

                blk = in_pool.tile([P, Wg * SUBW], FP8, tag="blk")
                nc.sync.dma_start(out=blk[:],
                                  in_=edge8[:, s0 * SUBW:(s0 + Wg) * SUBW])

                # consumers whose inputs are >=1 iteration old come first so
                # every engine starts its iteration with ready work
                if stD is not None:
                    stage_D(stD)
                while pending_epi:
                    epilogue(pending_epi.pop(0))

                kve = kve_pool.tile([P, GROUP, 2 * P], F32, tag="kve")
                qd = qd_pool.tile([P, GROUP, P], F32, tag="qd")
                for j in range(Wg):
                    xd_j = blk[:, j * SUBW + 3 * P:(j + 1) * SUBW].bitcast(BF16)
                    nc.tensor.matmul(out=qd[:, j, :],
                                     lhsT=xd_j,
                                     rhs=wsb["wq"][:], start=True,
                                     stop=not has_bias, skip_group_check=True)
                    if has_bias:
                        nc.tensor.matmul(out=qd[:, j, :], lhsT=ones_row[:],
                                         rhs=bq_sb[:], start=False, stop=True,
                                         skip_group_check=True)
                for j in range(Wg):
                    kvp = blk[:, j * SUBW:j * SUBW + 2 * P].rearrange(
                        "p (two m) -> p two m", two=2)
                    nc.tensor.matmul(
                        out=kve[:, j, :], lhsT=kvp, rhs=w2_sb[:],
                        start=True, stop=not has_bias,
                        perf_mode=mybir.MatmulPerfMode.DoubleRow,
                        skip_group_check=True)
                    if has_bias:
                        nc.tensor.matmul(out=kve[:, j, :], lhsT=ones_row[:],
                                         rhs=bkv_sb[:], start=False, stop=True,
                                         skip_group_check=True)

                qd_sb = wk_pool.tile([P, Wg, P], BF16, tag="qd_sb")
                nc.scalar.copy(out=qd_sb[:], in_=qd[:, 0:Wg, :])

                if stC is not None:
                    stage_C(stC)

                stD = stC
                stC = dict(w=w, s0=s0, Wg=Wg, kve=kve, qd_sb=qd_sb, blk=blk)

            stage_C(stC)
            stage_D(stD)
            stage_D(stC)
            while pending_epi:
                epilogue(pending_epi.pop(0))

    nc.compile()
    return nc


# ----------------------------------------------------------------------------
# entry point
# ----------------------------------------------------------------------------

def kernel(**inputs):
    global LAST_EXEC_TIME_NS, LAST_RESULTS
    assert np.asarray(inputs['x']).shape == (N, DIM)
    assert np.asarray(inputs['edge_index']).shape == (2, E)

    sched, in_maps, has_bias, node_row = _device_inputs(inputs)
    nc = _build(sched, has_bias=has_bias)
    res = bass_utils.run_bass_kernel_spmd(
        nc, in_maps, core_ids=list(range(NCORES)), trace=TRACE)
    LAST_EXEC_TIME_NS = res.exec_time_ns
    LAST_RESULTS = res
    rows = np.concatenate([r['out'] for r in res.results], axis=0)
    return np.ascontiguousarray(rows[node_row].astype(np.float32))


# revision 42
# speedup vs baseline: 1.1557x; 1.1167x over previous
"""TransformerConv GNN message passing on 8 TRN2 NeuronCores (Bass/Tile).

Strategy (graph/edge parallelism, dst-sharded - no collectives needed):
  - Core c owns destination nodes [c*6250, (c+1)*6250); edges are sharded by
    their dst node, so the segment-softmax and scatter-aggregation are fully
    core-local.
  - Per the sharding hint, edges ship with their GATHERED node features:
    the host packs x[src], edge_attr (paired fp8 K-tiles for a DoubleRow
    matmul) and x[dst] (fp8) per 128-edge sub-chunk into one fused stream,
    plus a tiny f32 stream of window-local dst indices (one column per
    sub-chunk; -1 for padding edges).
  - On device, per dst-window of 128 nodes, per group of <=4 sub-chunks:
      kve = DoubleRow fp8 matmul: xs@[Wk|Wv] + ea@[We|We]  (one instr/chunk)
      qd  = xd@Wq (fp8)                                    (PE)
      oh  = is_equal(iota_row, dst_col)                    (DVE, replaces the
                                                            shipped onehot)
      alpha = rowsum_per_head(qd_sb * kve.k)               (DVE, bf16 out)
      pe  = exp(alpha/8)                                   (ACT, max-shift
                                                            dropped: identical)
      ve  = kve.v * pe_broadcast                           (Pool/GpSimd)
      [ve | pe] scatter: agg[128,130] += oh.T @ ve         (PE)
    Window epilogue: out = (agg/denom) @ Wproj + x_own @ (Wskip@Wproj) + bias.
  - Softmax normalization is applied after aggregation (linearity); padding
    edges carry dst=-1 so their generated onehot row is all-zero.

kernel(**inputs) takes the FULL unsharded inputs and returns the FULL
[50000, 128] float32 output.  Set TRACE=True to capture NTFF timing
(LAST_EXEC_TIME_NS / LAST_RESULTS are populated).
"""
import sys
from contextlib import ExitStack

import numpy as np

for _p in ('/opt/trn_rl_repo', '/root/.axon_site/_ro/trn_rl_repo'):
    if _p not in sys.path:
        sys.path.append(_p)

import ml_dtypes

import concourse.bass as bass          # noqa: E402
import concourse.mybir as mybir        # noqa: E402
import concourse.tile as tile          # noqa: E402
from concourse import bacc             # noqa: E402
from concourse import bass_utils       # noqa: E402

bf16 = ml_dtypes.bfloat16
f8 = ml_dtypes.float8_e4m3
F32 = mybir.dt.float32
BF16 = mybir.dt.bfloat16
FP8 = mybir.dt.float8e4

N = 50000
E = 800000
DIM = 128
H = 2
C = 64
P = 128
NCORES = 8
NODES_PER_CORE = N // NCORES          # 6250
WIN = 128
NWIN = (NODES_PER_CORE + WIN - 1) // WIN   # 49
NODES_PAD = NWIN * WIN                # 6272
GROUP = 4
ALPHA_SCALE = 0.125                   # 1/sqrt(64)
SUBW = 5 * P                          # fp8-byte cols per sub-chunk: xs|ea|oh|xd(bf16 as 2B)

TRACE = False
LAST_EXEC_TIME_NS = None
LAST_RESULTS = None


# ----------------------------------------------------------------------------
# host-side sharding / preprocessing
# ----------------------------------------------------------------------------

def _schedule(S):
    groups = []
    sub_base = 0
    for w in range(NWIN):
        for g0 in range(0, S[w], GROUP):
            Wg = min(GROUP, S[w] - g0)
            groups.append((w, sub_base + g0, Wg))
        sub_base += S[w]
    return groups, sub_base


def _balance(dst):
    """Greedy balanced assignment of nodes to (core, window) bins.

    Returns (bin_of[N], slot_of[N]): bin b holds exactly WIN nodes; node n sits
    at window-local slot slot_of[n].  Bins are load-balanced by in-degree so
    every bin has ~E/(NCORES*NWIN) incoming edges, minimizing sub-chunk padding.
    """
    import heapq
    deg = np.bincount(dst, minlength=N)
    NB = NCORES * NWIN
    order = np.argsort(-deg, kind='stable')
    heap = [(0, b) for b in range(NB)]
    heapq.heapify(heap)
    slots = np.zeros(NB, np.int32)
    bin_of = np.empty(N, np.int32)
    slot_of = np.empty(N, np.int32)
    for n in order:
        while True:
            load, b = heapq.heappop(heap)
            if slots[b] < WIN:
                break
        bin_of[n] = b
        slot_of[n] = slots[b]
        slots[b] += 1
        if slots[b] < WIN:
            heapq.heappush(heap, (load + int(deg[n]), b))
    return bin_of, slot_of


def _prep(x, edge_attr, edge_index):
    x_np = np.asarray(x, dtype=np.float32)
    src = np.asarray(edge_index[0], dtype=np.int64)
    dst = np.asarray(edge_index[1], dtype=np.int64)

    bin_of, slot_of = _balance(dst)
    core_of = (bin_of // NWIN)[dst]
    win_of = (bin_of % NWIN)[dst]
    dst_slot = slot_of[dst]

    counts = np.zeros((NCORES, NWIN), dtype=np.int64)
    np.add.at(counts, (core_of, win_of), 1)
    S = np.maximum(np.ceil(counts / P).astype(np.int64).max(axis=0), 1)
    TS = int(S.sum())
    EPAD = TS * P

    order = np.lexsort((np.arange(E), win_of, core_of))
    run_ends = np.cumsum(counts.reshape(-1))
    run_starts = np.concatenate([[0], run_ends[:-1]]).reshape(NCORES, NWIN)
    run_ends = run_ends.reshape(NCORES, NWIN)

    ea_np = np.asarray(edge_attr, dtype=np.float32)
    wbase = np.concatenate([[0], np.cumsum(S)])
    per_core = []
    for c in range(NCORES):
        src_pad = np.zeros(EPAD, dtype=np.int64)
        dstg_pad = np.zeros(EPAD, dtype=np.int64)
        dstoh_pad = np.full(EPAD, -1, dtype=np.int64)
        ea_rows = np.zeros(EPAD, dtype=np.int64)
        ea_valid = np.zeros(EPAD, dtype=bool)
        for w in range(NWIN):
            sel = order[run_starts[c, w]:run_ends[c, w]]
            cnt = len(sel)
            base = int(wbase[w]) * P
            src_pad[base:base + cnt] = src[sel]
            dstg_pad[base:base + cnt] = dst[sel]
            dstoh_pad[base:base + cnt] = dst_slot[sel]
            ea_rows[base:base + cnt] = sel
            ea_valid[base:base + cnt] = True

        ea = np.zeros((EPAD, DIM), dtype=np.float32)
        ea[ea_valid] = ea_np[ea_rows[ea_valid]]
        xs = x_np[src_pad]
        xs[~ea_valid] = 0.0
        xd = x_np[dstg_pad]
        oh = np.zeros((EPAD, P), dtype=np.float32)   # [edges, window-nodes]
        vmask = dstoh_pad >= 0
        oh[np.nonzero(vmask)[0], dstoh_pad[vmask]] = 1.0

        def sub_t(mat):   # feature dim on partitions, per 128-edge sub-chunk
            return np.ascontiguousarray(
                mat.reshape(TS, P, P).transpose(2, 0, 1)).reshape(P, EPAD)

        def sub_n(mat):   # edges on partitions (onehot)
            return np.ascontiguousarray(
                mat.reshape(TS, P, P).transpose(1, 0, 2)).reshape(P, EPAD)

        xsT = sub_t(xs).astype(f8)
        eaT = sub_t(ea).astype(f8)
        ohN = sub_n(oh).astype(f8)
        xdT = sub_t(xd).astype(bf16)
        xdB = np.ascontiguousarray(xdT).view(np.uint8)   # [P, TS*P*2] raw bytes

        edge8 = np.empty((P, TS * SUBW), dtype=f8)
        e8u = edge8.view(np.uint8)
        for s in range(TS):
            o = s * SUBW
            edge8[:, o:o + P] = xsT[:, s * P:(s + 1) * P]
            edge8[:, o + P:o + 2 * P] = eaT[:, s * P:(s + 1) * P]
            edge8[:, o + 2 * P:o + 3 * P] = ohN[:, s * P:(s + 1) * P]
            e8u[:, o + 3 * P:o + 5 * P] = xdB[:, s * 2 * P:(s + 1) * 2 * P]

        per_core.append(edge8)

    # global output row of each node: core*NODES_PAD + window*P + slot
    node_row = ((bin_of // NWIN).astype(np.int64) * NODES_PAD
                + (bin_of % NWIN).astype(np.int64) * P
                + slot_of.astype(np.int64))
    return per_core, dict(S=S.tolist(), TS=TS), node_row


def _device_inputs(inputs):
    x = np.asarray(inputs['x'], dtype=np.float32)
    per_core, sched, node_row = _prep(x, inputs['edge_attr'],
                                      inputs['edge_index'])
    ident = np.eye(P, dtype=np.float32).astype(bf16)
    biases = {k: np.asarray(inputs[k], dtype=np.float32)
              for k in ['bq', 'bk', 'bv', 'bskip', 'bproj']}
    has_bias = any(np.any(b != 0) for b in biases.values())

    wk = np.asarray(inputs['Wk'], dtype=np.float32)
    wv = np.asarray(inputs['Wv'], dtype=np.float32)
    we = np.asarray(inputs['We'], dtype=np.float32)
    w2f = np.empty((P, 2, 2 * P), dtype=np.float32)  # K-tile 0: [Wk|Wv], 1: [We|We]
    w2f[:, 0, 0:P] = wk
    w2f[:, 0, P:2 * P] = wv
    w2f[:, 1, 0:P] = we
    w2f[:, 1, P:2 * P] = we
    w2 = w2f.astype(f8)

    # xTown rows follow the balanced (core, window, slot) node layout
    xext = np.zeros((NCORES * NODES_PAD, DIM), dtype=np.float32)
    xext[node_row] = x

    in_maps = []
    for c in range(NCORES):
        own = xext[c * NODES_PAD:(c + 1) * NODES_PAD]
        m = dict(
            edge8=per_core[c],
            xTown_pm=np.ascontiguousarray(own.T).astype(bf16),
            ident_in=ident,
            w2_pm=w2,
            wq=np.asarray(inputs['Wq'], dtype=np.float32),
            wskip=np.asarray(inputs['Wskip'], dtype=np.float32),
            wproj=np.asarray(inputs['Wproj'], dtype=np.float32),
        )
        if has_bias:
            m['bkv_row'] = np.ascontiguousarray(
                np.concatenate([biases['bk'], biases['bv']])[None, :])
            m['bq_row'] = np.ascontiguousarray(biases['bq'][None, :])
            m['bskip_col'] = np.ascontiguousarray(biases['bskip'][:, None])
            m['bproj_row'] = np.ascontiguousarray(biases['bproj'][None, :])
        in_maps.append(m)
    return sched, in_maps, has_bias, node_row


# ----------------------------------------------------------------------------
# device kernel
# ----------------------------------------------------------------------------

def _build(sched, has_bias=False):
    S = sched['S']
    groups, TS = _schedule(S)
    nc = bacc.Bacc("TRN2", target_bir_lowering=False, debug=False)

    edge8 = nc.dram_tensor("edge8", [P, TS * SUBW], FP8, kind="ExternalInput").ap()
    xTown_pm = nc.dram_tensor("xTown_pm", [P, NODES_PAD], BF16, kind="ExternalInput").ap()
    ident_in = nc.dram_tensor("ident_in", [P, P], BF16, kind="ExternalInput").ap()
    w2_pm = nc.dram_tensor("w2_pm", [P, 2, 2 * P], FP8, kind="ExternalInput").ap()
    w_in = {}
    for name in ["wq", "wskip", "wproj"]:
        w_in[name] = nc.dram_tensor(name, [P, P], F32, kind="ExternalInput").ap()
    if has_bias:
        bkv_row = nc.dram_tensor("bkv_row", [1, 2 * P], F32, kind="ExternalInput").ap()
        bq_row = nc.dram_tensor("bq_row", [1, P], F32, kind="ExternalInput").ap()
        bskip_col = nc.dram_tensor("bskip_col", [P, 1], F32, kind="ExternalInput").ap()
        bproj_row = nc.dram_tensor("bproj_row", [1, P], F32, kind="ExternalInput").ap()
    out = nc.dram_tensor("out", [NODES_PAD, DIM], F32, kind="ExternalOutput").ap()

    with tile.TileContext(nc) as tc, ExitStack() as top:
        res = top.enter_context(tc.tile_pool(name="res", bufs=1))

        xTown_sb = res.tile([P, NODES_PAD], BF16)
        nc.sync.dma_start(out=xTown_sb[:], in_=xTown_pm[:, :])
        ident = res.tile([P, P], BF16)
        nc.sync.dma_start(out=ident[:], in_=ident_in[:, :])
        w2_sb = res.tile([P, 2, 2 * P], FP8)
        nc.sync.dma_start(out=w2_sb[:], in_=w2_pm[:, :, :])

        wsb = {}
        for name in ["wq", "wskip", "wproj"]:
            wf = res.tile([P, P], F32, tag="wf32")
            nc.sync.dma_start(out=wf[:], in_=w_in[name][:, :])
            wb = res.tile([P, P], BF16, tag=f"{name}_b")
            nc.vector.tensor_copy(out=wb[:], in_=wf[:])
            wsb[name] = wb

        if has_bias:
            bkv_sb = res.tile([1, 2 * P], BF16)
            bq_sb = res.tile([1, P], BF16)
            ones_row = res.tile([1, P], BF16)
            nc.vector.memset(ones_row[:], 1.0)
            bkvf = res.tile([1, 2 * P], F32)
            nc.sync.dma_start(out=bkvf[:], in_=bkv_row[:, :])
            nc.vector.tensor_copy(out=bkv_sb[:], in_=bkvf[:])
            bqf = res.tile([1, P], F32)
            nc.sync.dma_start(out=bqf[:], in_=bq_row[:, :])
            nc.vector.tensor_copy(out=bq_sb[:], in_=bqf[:])
            bskipc = res.tile([P, 1], F32)
            nc.sync.dma_start(out=bskipc[:], in_=bskip_col[:, :])
            bskipc_b = res.tile([P, 1], BF16)
            nc.vector.tensor_copy(out=bskipc_b[:], in_=bskipc[:])
            bprojf = res.tile([1, P], F32)
            nc.sync.dma_start(out=bprojf[:], in_=bproj_row[:, :])

        # fused skip weight: Wfused = Wskip @ Wproj  (and fused bias)
        wfused_sb = res.tile([P, P], BF16)
        bfused_sb = res.tile([1, P], BF16, name="bfused_sb") if has_bias else None
        with tc.tile_pool(name="wset_ps", bufs=1, space="PSUM") as wps_pool, \
             tc.tile_pool(name="wset_sb", bufs=1) as wsb_pool:
            tp = wps_pool.tile([P, P], BF16)
            nc.tensor.transpose(out=tp[:], in_=wsb["wskip"][:], identity=ident[:])
            wskipT = wsb_pool.tile([P, P], BF16)
            nc.vector.tensor_copy(out=wskipT[:], in_=tp[:])
            wf_ps = wps_pool.tile([P, P], F32)
            nc.tensor.matmul(out=wf_ps[:], lhsT=wskipT[:], rhs=wsb["wproj"][:],
                             start=True, stop=True)
            nc.vector.tensor_copy(out=wfused_sb[:], in_=wf_ps[:])
            if has_bias:
                bf_ps = wps_pool.tile([1, P], F32)
                nc.tensor.matmul(out=bf_ps[:], lhsT=bskipc_b[:], rhs=wsb["wproj"][:],
                                 start=True, stop=True)
                bff = wsb_pool.tile([1, P], F32)
                nc.vector.tensor_add(out=bff[:], in0=bf_ps[:], in1=bprojf[:])
                nc.vector.tensor_copy(out=bfused_sb[:], in_=bff[:])

        # ---------------- main loop (3-stage software pipeline) -------------
        with tc.tile_pool(name="edge_in", bufs=12) as in_pool, \
             tc.tile_pool(name="work", bufs=10) as wk_pool, \
             tc.tile_pool(name="kve_ps", bufs=3, space="PSUM") as kve_pool, \
             tc.tile_pool(name="qd_ps", bufs=1, space="PSUM") as qd_pool, \
             tc.tile_pool(name="agg_ps", bufs=1, space="PSUM") as agg_pool, \
             tc.tile_pool(name="outp", bufs=8) as out_pool:
            aggs = {}

            def epilogue(w):
                agg = aggs.pop(w)
                den = out_pool.tile([P, H], F32, tag="den", name=f"den{w}")
                nc.vector.tensor_scalar_add(den[:], agg[:, P:P + H], 1e-30)
                inv = out_pool.tile([P, H], F32, tag="inv", name=f"inv{w}")
                nc.vector.reciprocal(out=inv[:], in_=den[:])
                aggn = out_pool.tile([P, P], BF16, tag="aggn", name=f"aggn{w}")
                nc.vector.tensor_mul(
                    out=aggn[:].rearrange("p (h c) -> p h c", c=C),
                    in0=agg[:, 0:P].rearrange("p (h c) -> p h c", c=C),
                    in1=inv[:].unsqueeze(2).broadcast_to([P, H, C]))
                tp_ps = agg_pool.tile([P, P], BF16, tag="agg", name=f"tp{w}")
                nc.tensor.transpose(out=tp_ps[:], in_=aggn[:], identity=ident[:])
                aggT = out_pool.tile([P, P], BF16, tag="aggT", name=f"aggT{w}")
                nc.scalar.copy(out=aggT[:], in_=tp_ps[:])
                fin = agg_pool.tile([P, P], F32, tag="agg", name=f"fin{w}")
                nc.tensor.matmul(out=fin[:], lhsT=aggT[:], rhs=wsb["wproj"][:],
                                 start=True, stop=False, skip_group_check=True)
                nc.tensor.matmul(out=fin[:], lhsT=xTown_sb[:, w * P:(w + 1) * P],
                                 rhs=wfused_sb[:], start=False,
                                 stop=not has_bias, skip_group_check=True)
                if has_bias:
                    nc.tensor.matmul(out=fin[:], lhsT=ones_row[:], rhs=bfused_sb[:],
                                     start=False, stop=True, skip_group_check=True)
                fin_sb = out_pool.tile([P, P], F32, tag="fin_sb", name=f"fsb{w}")
                nc.scalar.copy(out=fin_sb[:], in_=fin[:])
                nc.sync.dma_start(out=out[w * P:(w + 1) * P, :], in_=fin_sb[:])

            def stage_C(st):
                Wg = st['Wg']
                qk = wk_pool.tile([P, Wg, P], BF16, tag="qk", name=f"qk{st['s0']}")
                nc.vector.tensor_mul(out=qk[:], in0=st['qd_sb'][:],
                                     in1=st['kve'][:, 0:Wg, 0:P])
                alpha = wk_pool.tile([P, Wg, H], F32, tag="alpha",
                                     name=f"al{st['s0']}")
                nc.vector.reduce_sum(
                    out=alpha[:],
                    in_=qk[:].rearrange("p j (h c) -> p (j h) c", c=C),
                    axis=mybir.AxisListType.X)
                pe = wk_pool.tile([P, Wg, H], BF16, tag="pe", name=f"pe{st['s0']}")
                nc.scalar.activation(
                    out=pe[:], in_=alpha[:],
                    func=mybir.ActivationFunctionType.Exp, scale=ALPHA_SCALE)
                st['alpha'] = alpha
                st['pe'] = pe

            def stage_D(st):
                Wg = st['Wg']
                w, s0 = st['w'], st['s0']
                ve = wk_pool.tile([P, Wg, 130], BF16, tag="ve", name=f"ve{s0}")
                nc.vector.tensor_mul(
                    out=ve[:, :, 0:P].rearrange("p j (h c) -> p j h c", c=C),
                    in0=st['kve'][:, 0:Wg, P:2 * P].rearrange(
                        "p j (h c) -> p j h c", c=C),
                    in1=st['pe'][:].unsqueeze(3).broadcast_to([P, Wg, H, C]))
                nc.scalar.activation(
                    out=ve[:, :, P:P + H], in_=st['alpha'][:],
                    func=mybir.ActivationFunctionType.Exp, scale=ALPHA_SCALE)
                Sw = S[w]
                wstart = sum(S[:w])
                blk = st['blk']
                for j in range(Wg):
                    nd = s0 - wstart + j
                    nc.tensor.matmul(
                        out=aggs[w][:],
                        lhsT=blk[:, j * SUBW + 2 * P:j * SUBW + 3 * P],
                        rhs=ve[:, j, :],
                        start=(nd == 0), stop=(nd == Sw - 1),
                        skip_group_check=True)
                if s0 - wstart + Wg == Sw:
                    pending_epi.append((w, 1))

            stC = None
            stD = None
            cur_w = -1
            pending_epi = []
            for (w, s0, Wg) in groups:
                if w != cur_w:
                    cur_w = w
                    aggs[w] = agg_pool.tile([P, 130], F32, tag="agg", name=f"agg{w}")

                blk = in_pool.tile([P, Wg * SUBW], FP8, tag="blk")
                nc.sync.dma_start(out=blk[:],
                                  in_=edge8[:, s0 * SUBW:(s0 + Wg) * SUBW])

                # consumers whose inputs are >=1 iteration old come first so
                # every engine starts its iteration with ready work
                if stD is not None:
                    stage_D(stD)
                # run epilogues two iterations after their last scatter so the
                # den/aggn DVE ops never wait on an in-flight PE round-trip
                while pending_epi and pending_epi[0][1] <= 0:
                    epilogue(pending_epi.pop(0)[0])
                pending_epi = [(pw, lag - 1) for (pw, lag) in pending_epi]

                kve = kve_pool.tile([P, GROUP, 2 * P], F32, tag="kve")
                qd = qd_pool.tile([P, GROUP, P], F32, tag="qd")
                for j in range(Wg):
                    xd_j = blk[:, j * SUBW + 3 * P:(j + 1) * SUBW].bitcast(BF16)
                    nc.tensor.matmul(out=qd[:, j, :],
                                     lhsT=xd_j,
                                     rhs=wsb["wq"][:], start=True,
                                     stop=not has_bias, skip_group_check=True)
                    if has_bias:
                        nc.tensor.matmul(out=qd[:, j, :], lhsT=ones_row[:],
                                         rhs=bq_sb[:], start=False, stop=True,
                                         skip_group_check=True)
                for j in range(Wg):
                    kvp = blk[:, j * SUBW:j * SUBW + 2 * P].rearrange(
                        "p (two m) -> p two m", two=2)
                    nc.tensor.matmul(
                        out=kve[:, j, :], lhsT=kvp, rhs=w2_sb[:],
                        start=True, stop=not has_bias,
                        perf_mode=mybir.MatmulPerfMode.DoubleRow,
                        skip_group_check=True)
                    if has_bias:
                        nc.tensor.matmul(out=kve[:, j, :], lhsT=ones_row[:],
                                         rhs=bkv_sb[:], start=False, stop=True,
                                         skip_group_check=True)

                qd_sb = wk_pool.tile([P, Wg, P], BF16, tag="qd_sb")
                nc.scalar.copy(out=qd_sb[:], in_=qd[:, 0:Wg, :])

                if stC is not None:
                    stage_C(stC)

                stD = stC
                stC = dict(w=w, s0=s0, Wg=Wg, kve=kve, qd_sb=qd_sb, blk=blk)

            stage_C(stC)
            stage_D(stD)
            stage_D(stC)
            while pending_epi:
                epilogue(pending_epi.pop(0)[0])

    nc.compile()
    return nc


# ----------------------------------------------------------------------------
# entry point
# ----------------------------------------------------------------------------

def kernel(**inputs):
    global LAST_EXEC_TIME_NS, LAST_RESULTS
    assert np.asarray(inputs['x']).shape == (N, DIM)
    assert np.asarray(inputs['edge_index']).shape == (2, E)

    sched, in_maps, has_bias, node_row = _device_inputs(inputs)
    nc = _build(sched, has_bias=has_bias)
    res = bass_utils.run_bass_kernel_spmd(
        nc, in_maps, core_ids=list(range(NCORES)), trace=TRACE)
    LAST_EXEC_TIME_NS = res.exec_time_ns
    LAST_RESULTS = res
    rows = np.concatenate([r['out'] for r in res.results], axis=0)
    return np.ascontiguousarray(rows[node_row].astype(np.float32))


# revision 45
# speedup vs baseline: 1.2718x; 1.1004x over previous
"""TransformerConv GNN message passing on 8 TRN2 NeuronCores (Bass/Tile).

Strategy (graph/edge parallelism, dst-sharded - no collectives needed):
  - Core c owns destination nodes [c*6250, (c+1)*6250); edges are sharded by
    their dst node, so the segment-softmax and scatter-aggregation are fully
    core-local.
  - Per the sharding hint, edges ship with their GATHERED node features:
    the host packs x[src], edge_attr (paired fp8 K-tiles for a DoubleRow
    matmul) and x[dst] (fp8) per 128-edge sub-chunk into one fused stream,
    plus a tiny f32 stream of window-local dst indices (one column per
    sub-chunk; -1 for padding edges).
  - On device, per dst-window of 128 nodes, per group of <=4 sub-chunks:
      kve = DoubleRow fp8 matmul: xs@[Wk|Wv] + ea@[We|We]  (one instr/chunk)
      qd  = xd@Wq (fp8)                                    (PE)
      oh  = is_equal(iota_row, dst_col)                    (DVE, replaces the
                                                            shipped onehot)
      alpha = rowsum_per_head(qd_sb * kve.k)               (DVE, bf16 out)
      pe  = exp(alpha/8)                                   (ACT, max-shift
                                                            dropped: identical)
      ve  = kve.v * pe_broadcast                           (Pool/GpSimd)
      [ve | pe] scatter: agg[128,130] += oh.T @ ve         (PE)
    Window epilogue: out = (agg/denom) @ Wproj + x_own @ (Wskip@Wproj) + bias.
  - Softmax normalization is applied after aggregation (linearity); padding
    edges carry dst=-1 so their generated onehot row is all-zero.

kernel(**inputs) takes the FULL unsharded inputs and returns the FULL
[50000, 128] float32 output.  Set TRACE=True to capture NTFF timing
(LAST_EXEC_TIME_NS / LAST_RESULTS are populated).
"""
import sys
from contextlib import ExitStack

import numpy as np

for _p in ('/opt/trn_rl_repo', '/root/.axon_site/_ro/trn_rl_repo'):
    if _p not in sys.path:
        sys.path.append(_p)

import ml_dtypes

import concourse.bass as bass          # noqa: E402
import concourse.mybir as mybir        # noqa: E402
import concourse.tile as tile          # noqa: E402
from concourse import bacc             # noqa: E402
from concourse import bass_utils       # noqa: E402

bf16 = ml_dtypes.bfloat16
f8 = ml_dtypes.float8_e4m3
F32 = mybir.dt.float32
BF16 = mybir.dt.bfloat16
FP8 = mybir.dt.float8e4

N = 50000
E = 800000
DIM = 128
H = 2
C = 64
P = 128
NCORES = 8
NODES_PER_CORE = N // NCORES          # 6250
WIN = 128
NWIN = (NODES_PER_CORE + WIN - 1) // WIN   # 49
NODES_PAD = NWIN * WIN                # 6272
GROUP = 4
ALPHA_SCALE = 0.125                   # 1/sqrt(64)
SUBW = 5 * P                          # fp8-byte cols per sub-chunk: xs|ea|oh|xd(bf16 as 2B)

TRACE = False
LAST_EXEC_TIME_NS = None
LAST_RESULTS = None


# ----------------------------------------------------------------------------
# host-side sharding / preprocessing
# ----------------------------------------------------------------------------

def _schedule(S):
    groups = []
    sub_base = 0
    for w in range(NWIN):
        for g0 in range(0, S[w], GROUP):
            Wg = min(GROUP, S[w] - g0)
            groups.append((w, sub_base + g0, Wg))
        sub_base += S[w]
    return groups, sub_base


def _balance(dst):
    """Greedy balanced assignment of nodes to (core, window) bins.

    Returns (bin_of[N], slot_of[N]): bin b holds exactly WIN nodes; node n sits
    at window-local slot slot_of[n].  Bins are load-balanced by in-degree so
    every bin has ~E/(NCORES*NWIN) incoming edges, minimizing sub-chunk padding.
    """
    import heapq
    deg = np.bincount(dst, minlength=N)
    NB = NCORES * NWIN
    order = np.argsort(-deg, kind='stable')
    heap = [(0, b) for b in range(NB)]
    heapq.heapify(heap)
    slots = np.zeros(NB, np.int32)
    bin_of = np.empty(N, np.int32)
    slot_of = np.empty(N, np.int32)
    for n in order:
        while True:
            load, b = heapq.heappop(heap)
            if slots[b] < WIN:
                break
        bin_of[n] = b
        slot_of[n] = slots[b]
        slots[b] += 1
        if slots[b] < WIN:
            heapq.heappush(heap, (load + int(deg[n]), b))
    return bin_of, slot_of


def _prep(x, edge_attr, edge_index):
    x_np = np.asarray(x, dtype=np.float32)
    src = np.asarray(edge_index[0], dtype=np.int64)
    dst = np.asarray(edge_index[1], dtype=np.int64)

    bin_of, slot_of = _balance(dst)
    core_of = (bin_of // NWIN)[dst]
    win_of = (bin_of % NWIN)[dst]
    dst_slot = slot_of[dst]

    counts = np.zeros((NCORES, NWIN), dtype=np.int64)
    np.add.at(counts, (core_of, win_of), 1)
    S = np.maximum(np.ceil(counts / P).astype(np.int64).max(axis=0), 1)
    TS = int(S.sum())
    EPAD = TS * P

    order = np.lexsort((np.arange(E), win_of, core_of))
    run_ends = np.cumsum(counts.reshape(-1))
    run_starts = np.concatenate([[0], run_ends[:-1]]).reshape(NCORES, NWIN)
    run_ends = run_ends.reshape(NCORES, NWIN)

    ea_np = np.asarray(edge_attr, dtype=np.float32)
    wbase = np.concatenate([[0], np.cumsum(S)])
    per_core = []
    for c in range(NCORES):
        src_pad = np.zeros(EPAD, dtype=np.int64)
        dstg_pad = np.zeros(EPAD, dtype=np.int64)
        dstoh_pad = np.full(EPAD, -1, dtype=np.int64)
        ea_rows = np.zeros(EPAD, dtype=np.int64)
        ea_valid = np.zeros(EPAD, dtype=bool)
        for w in range(NWIN):
            sel = order[run_starts[c, w]:run_ends[c, w]]
            cnt = len(sel)
            base = int(wbase[w]) * P
            src_pad[base:base + cnt] = src[sel]
            dstg_pad[base:base + cnt] = dst[sel]
            dstoh_pad[base:base + cnt] = dst_slot[sel]
            ea_rows[base:base + cnt] = sel
            ea_valid[base:base + cnt] = True

        ea = np.zeros((EPAD, DIM), dtype=np.float32)
        ea[ea_valid] = ea_np[ea_rows[ea_valid]]
        xs = x_np[src_pad]
        xs[~ea_valid] = 0.0
        xd = x_np[dstg_pad]
        oh = np.zeros((EPAD, P), dtype=np.float32)   # [edges, window-nodes]
        vmask = dstoh_pad >= 0
        oh[np.nonzero(vmask)[0], dstoh_pad[vmask]] = 1.0

        def sub_t(mat):   # feature dim on partitions, per 128-edge sub-chunk
            return np.ascontiguousarray(
                mat.reshape(TS, P, P).transpose(2, 0, 1)).reshape(P, EPAD)

        def sub_n(mat):   # edges on partitions (onehot)
            return np.ascontiguousarray(
                mat.reshape(TS, P, P).transpose(1, 0, 2)).reshape(P, EPAD)

        xsT = sub_t(xs).astype(f8)
        eaT = sub_t(ea).astype(f8)
        ohN = sub_n(oh).astype(f8)
        xdT = sub_t(xd).astype(bf16)
        xdB = np.ascontiguousarray(xdT).view(np.uint8)   # [P, TS*P*2] raw bytes

        edge8 = np.empty((P, TS * SUBW), dtype=f8)
        e8u = edge8.view(np.uint8)
        for s in range(TS):
            o = s * SUBW
            edge8[:, o:o + P] = xsT[:, s * P:(s + 1) * P]
            edge8[:, o + P:o + 2 * P] = eaT[:, s * P:(s + 1) * P]
            edge8[:, o + 2 * P:o + 3 * P] = ohN[:, s * P:(s + 1) * P]
            e8u[:, o + 3 * P:o + 5 * P] = xdB[:, s * 2 * P:(s + 1) * 2 * P]

        per_core.append(edge8)

    # global output row of each node: core*NODES_PAD + window*P + slot
    node_row = ((bin_of // NWIN).astype(np.int64) * NODES_PAD
                + (bin_of % NWIN).astype(np.int64) * P
                + slot_of.astype(np.int64))
    return per_core, dict(S=S.tolist(), TS=TS), node_row


def _device_inputs(inputs):
    x = np.asarray(inputs['x'], dtype=np.float32)
    per_core, sched, node_row = _prep(x, inputs['edge_attr'],
                                      inputs['edge_index'])
    ident = np.eye(P, dtype=np.float32).astype(bf16)
    biases = {k: np.asarray(inputs[k], dtype=np.float32)
              for k in ['bq', 'bk', 'bv', 'bskip', 'bproj']}
    has_bias = any(np.any(b != 0) for b in biases.values())

    wk = np.asarray(inputs['Wk'], dtype=np.float32)
    wv = np.asarray(inputs['Wv'], dtype=np.float32)
    we = np.asarray(inputs['We'], dtype=np.float32)
    w2f = np.empty((P, 2, 2 * P), dtype=np.float32)  # K-tile 0: [Wk|Wv], 1: [We|We]
    w2f[:, 0, 0:P] = wk
    w2f[:, 0, P:2 * P] = wv
    w2f[:, 1, 0:P] = we
    w2f[:, 1, P:2 * P] = we
    w2 = w2f.astype(f8)

    # xTown rows follow the balanced (core, window, slot) node layout
    xext = np.zeros((NCORES * NODES_PAD, DIM), dtype=np.float32)
    xext[node_row] = x

    in_maps = []
    for c in range(NCORES):
        own = xext[c * NODES_PAD:(c + 1) * NODES_PAD]
        m = dict(
            edge8=per_core[c],
            xTown_pm=np.ascontiguousarray(own.T).astype(bf16),
            ident_in=ident,
            w2_pm=w2,
            wq=np.asarray(inputs['Wq'], dtype=np.float32),
            wskip=np.asarray(inputs['Wskip'], dtype=np.float32),
            wproj=np.asarray(inputs['Wproj'], dtype=np.float32),
        )
        if has_bias:
            m['bkv_row'] = np.ascontiguousarray(
                np.concatenate([biases['bk'], biases['bv']])[None, :])
            m['bq_row'] = np.ascontiguousarray(biases['bq'][None, :])
            m['bskip_col'] = np.ascontiguousarray(biases['bskip'][:, None])
            m['bproj_row'] = np.ascontiguousarray(biases['bproj'][None, :])
        in_maps.append(m)
    return sched, in_maps, has_bias, node_row


# ----------------------------------------------------------------------------
# device kernel
# ----------------------------------------------------------------------------

def _build(sched, has_bias=False):
    S = sched['S']
    groups, TS = _schedule(S)
    nc = bacc.Bacc("TRN2", target_bir_lowering=False, debug=False)

    edge8 = nc.dram_tensor("edge8", [P, TS * SUBW], FP8, kind="ExternalInput").ap()
    xTown_pm = nc.dram_tensor("xTown_pm", [P, NODES_PAD], BF16, kind="ExternalInput").ap()
    ident_in = nc.dram_tensor("ident_in", [P, P], BF16, kind="ExternalInput").ap()
    w2_pm = nc.dram_tensor("w2_pm", [P, 2, 2 * P], FP8, kind="ExternalInput").ap()
    w_in = {}
    for name in ["wq", "wskip", "wproj"]:
        w_in[name] = nc.dram_tensor(name, [P, P], F32, kind="ExternalInput").ap()
    if has_bias:
        bkv_row = nc.dram_tensor("bkv_row", [1, 2 * P], F32, kind="ExternalInput").ap()
        bq_row = nc.dram_tensor("bq_row", [1, P], F32, kind="ExternalInput").ap()
        bskip_col = nc.dram_tensor("bskip_col", [P, 1], F32, kind="ExternalInput").ap()
        bproj_row = nc.dram_tensor("bproj_row", [1, P], F32, kind="ExternalInput").ap()
    out = nc.dram_tensor("out", [NODES_PAD, DIM], F32, kind="ExternalOutput").ap()

    with tile.TileContext(nc) as tc, ExitStack() as top:
        res = top.enter_context(tc.tile_pool(name="res", bufs=1))

        xTown_sb = res.tile([P, NODES_PAD], BF16)
        nc.sync.dma_start(out=xTown_sb[:], in_=xTown_pm[:, :])
        ident = res.tile([P, P], BF16)
        nc.sync.dma_start(out=ident[:], in_=ident_in[:, :])
        w2_sb = res.tile([P, 2, 2 * P], FP8)
        nc.sync.dma_start(out=w2_sb[:], in_=w2_pm[:, :, :])

        wsb = {}
        for name in ["wq", "wskip", "wproj"]:
            wf = res.tile([P, P], F32, tag="wf32")
            nc.sync.dma_start(out=wf[:], in_=w_in[name][:, :])
            wb = res.tile([P, P], BF16, tag=f"{name}_b")
            nc.vector.tensor_copy(out=wb[:], in_=wf[:])
            wsb[name] = wb

        if has_bias:
            bkv_sb = res.tile([1, 2 * P], BF16)
            bq_sb = res.tile([1, P], BF16)
            ones_row = res.tile([1, P], BF16)
            nc.vector.memset(ones_row[:], 1.0)
            bkvf = res.tile([1, 2 * P], F32)
            nc.sync.dma_start(out=bkvf[:], in_=bkv_row[:, :])
            nc.vector.tensor_copy(out=bkv_sb[:], in_=bkvf[:])
            bqf = res.tile([1, P], F32)
            nc.sync.dma_start(out=bqf[:], in_=bq_row[:, :])
            nc.vector.tensor_copy(out=bq_sb[:], in_=bqf[:])
            bskipc = res.tile([P, 1], F32)
            nc.sync.dma_start(out=bskipc[:], in_=bskip_col[:, :])
            bskipc_b = res.tile([P, 1], BF16)
            nc.vector.tensor_copy(out=bskipc_b[:], in_=bskipc[:])
            bprojf = res.tile([1, P], F32)
            nc.sync.dma_start(out=bprojf[:], in_=bproj_row[:, :])

        # fused skip weight: Wfused = Wskip @ Wproj  (and fused bias)
        wfused_sb = res.tile([P, P], BF16)
        bfused_sb = res.tile([1, P], BF16, name="bfused_sb") if has_bias else None
        with tc.tile_pool(name="wset_ps", bufs=1, space="PSUM") as wps_pool, \
             tc.tile_pool(name="wset_sb", bufs=1) as wsb_pool:
            tp = wps_pool.tile([P, P], BF16)
            nc.tensor.transpose(out=tp[:], in_=wsb["wskip"][:], identity=ident[:])
            wskipT = wsb_pool.tile([P, P], BF16)
            nc.vector.tensor_copy(out=wskipT[:], in_=tp[:])
            wf_ps = wps_pool.tile([P, P], F32)
            nc.tensor.matmul(out=wf_ps[:], lhsT=wskipT[:], rhs=wsb["wproj"][:],
                             start=True, stop=True)
            nc.vector.tensor_copy(out=wfused_sb[:], in_=wf_ps[:])
            if has_bias:
                bf_ps = wps_pool.tile([1, P], F32)
                nc.tensor.matmul(out=bf_ps[:], lhsT=bskipc_b[:], rhs=wsb["wproj"][:],
                                 start=True, stop=True)
                bff = wsb_pool.tile([1, P], F32)
                nc.vector.tensor_add(out=bff[:], in0=bf_ps[:], in1=bprojf[:])
                nc.vector.tensor_copy(out=bfused_sb[:], in_=bff[:])

        # ---------------- main loop (3-stage software pipeline) -------------
        with tc.tile_pool(name="edge_in", bufs=12) as in_pool, \
             tc.tile_pool(name="work", bufs=10) as wk_pool, \
             tc.tile_pool(name="kve_ps", bufs=3, space="PSUM") as kve_pool, \
             tc.tile_pool(name="qd_ps", bufs=1, space="PSUM") as qd_pool, \
             tc.tile_pool(name="agg_ps", bufs=1, space="PSUM") as agg_pool, \
             tc.tile_pool(name="outp", bufs=8) as out_pool:
            aggs = {}

            def epilogue(w, agg_sb):
                den = out_pool.tile([P, H], F32, tag="den", name=f"den{w}")
                nc.vector.tensor_scalar_add(den[:], agg_sb[:, P:P + H], 1e-30)
                inv = out_pool.tile([P, H], F32, tag="inv", name=f"inv{w}")
                nc.vector.reciprocal(out=inv[:], in_=den[:])
                aggn = out_pool.tile([P, P], BF16, tag="aggn", name=f"aggn{w}")
                nc.vector.tensor_mul(
                    out=aggn[:].rearrange("p (h c) -> p h c", c=C),
                    in0=agg_sb[:, 0:P].rearrange("p (h c) -> p h c", c=C),
                    in1=inv[:].unsqueeze(2).broadcast_to([P, H, C]))
                tp_ps = agg_pool.tile([P, P], BF16, tag="agg", name=f"tp{w}")
                nc.tensor.transpose(out=tp_ps[:], in_=aggn[:], identity=ident[:])
                aggT = out_pool.tile([P, P], BF16, tag="aggT", name=f"aggT{w}")
                nc.scalar.copy(out=aggT[:], in_=tp_ps[:])
                fin = agg_pool.tile([P, P], F32, tag="agg", name=f"fin{w}")
                nc.tensor.matmul(out=fin[:], lhsT=aggT[:], rhs=wsb["wproj"][:],
                                 start=True, stop=False, skip_group_check=True)
                nc.tensor.matmul(out=fin[:], lhsT=xTown_sb[:, w * P:(w + 1) * P],
                                 rhs=wfused_sb[:], start=False,
                                 stop=not has_bias, skip_group_check=True)
                if has_bias:
                    nc.tensor.matmul(out=fin[:], lhsT=ones_row[:], rhs=bfused_sb[:],
                                     start=False, stop=True, skip_group_check=True)
                fin_sb = out_pool.tile([P, P], F32, tag="fin_sb", name=f"fsb{w}")
                nc.scalar.copy(out=fin_sb[:], in_=fin[:])
                nc.sync.dma_start(out=out[w * P:(w + 1) * P, :], in_=fin_sb[:])

            def stage_C(st):
                Wg = st['Wg']
                qk = wk_pool.tile([P, Wg, P], BF16, tag="qk", name=f"qk{st['s0']}")
                nc.vector.tensor_mul(out=qk[:], in0=st['qd_sb'][:],
                                     in1=st['kve'][:, 0:Wg, 0:P])
                alpha = wk_pool.tile([P, Wg, H], F32, tag="alpha",
                                     name=f"al{st['s0']}")
                nc.vector.reduce_sum(
                    out=alpha[:],
                    in_=qk[:].rearrange("p j (h c) -> p (j h) c", c=C),
                    axis=mybir.AxisListType.X)
                pe = wk_pool.tile([P, Wg, H], BF16, tag="pe", name=f"pe{st['s0']}")
                nc.scalar.activation(
                    out=pe[:], in_=alpha[:],
                    func=mybir.ActivationFunctionType.Exp, scale=ALPHA_SCALE)
                st['alpha'] = alpha
                st['pe'] = pe

            def stage_D(st):
                Wg = st['Wg']
                w, s0 = st['w'], st['s0']
                ve = wk_pool.tile([P, Wg, 130], BF16, tag="ve", name=f"ve{s0}")
                nc.vector.tensor_mul(
                    out=ve[:, :, 0:P].rearrange("p j (h c) -> p j h c", c=C),
                    in0=st['kve'][:, 0:Wg, P:2 * P].rearrange(
                        "p j (h c) -> p j h c", c=C),
                    in1=st['pe'][:].unsqueeze(3).broadcast_to([P, Wg, H, C]))
                nc.scalar.activation(
                    out=ve[:, :, P:P + H], in_=st['alpha'][:],
                    func=mybir.ActivationFunctionType.Exp, scale=ALPHA_SCALE)
                Sw = S[w]
                wstart = sum(S[:w])
                blk = st['blk']
                for j in range(Wg):
                    nd = s0 - wstart + j
                    nc.tensor.matmul(
                        out=aggs[w][:],
                        lhsT=blk[:, j * SUBW + 2 * P:j * SUBW + 3 * P],
                        rhs=ve[:, j, :],
                        start=(nd == 0), stop=(nd == Sw - 1),
                        skip_group_check=True)
                if s0 - wstart + Wg == Sw:
                    # evacuate agg PSUM->SBUF immediately: the next window's
                    # first scatter WAR-waits only on this cheap ACT copy
                    agg = aggs.pop(w)
                    agg_sb = out_pool.tile([P, 130], F32, tag="agg_sb",
                                           name=f"asb{w}")
                    nc.scalar.copy(out=agg_sb[:], in_=agg[:])
                    pending_epi.append((w, agg_sb, 1))

            stC = None
            stD = None
            cur_w = -1
            pending_epi = []
            for (w, s0, Wg) in groups:
                if w != cur_w:
                    cur_w = w
                    aggs[w] = agg_pool.tile([P, 130], F32, tag="agg", name=f"agg{w}")

                blk = in_pool.tile([P, Wg * SUBW], FP8, tag="blk")
                nc.sync.dma_start(out=blk[:],
                                  in_=edge8[:, s0 * SUBW:(s0 + Wg) * SUBW])

                # consumers whose inputs are >=1 iteration old come first so
                # every engine starts its iteration with ready work
                if stD is not None:
                    stage_D(stD)
                # run epilogues two iterations after their last scatter so the
                # den/aggn DVE ops never wait on an in-flight PE round-trip
                while pending_epi and pending_epi[0][2] <= 0:
                    pw, pagg, _ = pending_epi.pop(0)
                    epilogue(pw, pagg)
                pending_epi = [(pw, pagg, lag - 1)
                               for (pw, pagg, lag) in pending_epi]

                kve = kve_pool.tile([P, GROUP, 2 * P], F32, tag="kve")
                qd = qd_pool.tile([P, GROUP, P], F32, tag="qd")
                for j in range(Wg):
                    xd_j = blk[:, j * SUBW + 3 * P:(j + 1) * SUBW].bitcast(BF16)
                    nc.tensor.matmul(out=qd[:, j, :],
                                     lhsT=xd_j,
                                     rhs=wsb["wq"][:], start=True,
                                     stop=not has_bias, skip_group_check=True)
                    if has_bias:
                        nc.tensor.matmul(out=qd[:, j, :], lhsT=ones_row[:],
                                         rhs=bq_sb[:], start=False, stop=True,
                                         skip_group_check=True)
                for j in range(Wg):
                    kvp = blk[:, j * SUBW:j * SUBW + 2 * P].rearrange(
                        "p (two m) -> p two m", two=2)
                    nc.tensor.matmul(
                        out=kve[:, j, :], lhsT=kvp, rhs=w2_sb[:],
                        start=True, stop=not has_bias,
                        perf_mode=mybir.MatmulPerfMode.DoubleRow,
                        skip_group_check=True)
                    if has_bias:
                        nc.tensor.matmul(out=kve[:, j, :], lhsT=ones_row[:],
                                         rhs=bkv_sb[:], start=False, stop=True,
                                         skip_group_check=True)

                qd_sb = wk_pool.tile([P, Wg, P], BF16, tag="qd_sb")
                nc.scalar.copy(out=qd_sb[:], in_=qd[:, 0:Wg, :])

                if stC is not None:
                    stage_C(stC)

                stD = stC
                stC = dict(w=w, s0=s0, Wg=Wg, kve=kve, qd_sb=qd_sb, blk=blk)

            stage_C(stC)
            stage_D(stD)
            stage_D(stC)
            while pending_epi:
                pw, pagg, _ = pending_epi.pop(0)
                epilogue(pw, pagg)

    nc.compile()
    return nc


# ----------------------------------------------------------------------------
# entry point
# ----------------------------------------------------------------------------

def kernel(**inputs):
    global LAST_EXEC_TIME_NS, LAST_RESULTS
    assert np.asarray(inputs['x']).shape == (N, DIM)
    assert np.asarray(inputs['edge_index']).shape == (2, E)

    sched, in_maps, has_bias, node_row = _device_inputs(inputs)
    nc = _build(sched, has_bias=has_bias)
    res = bass_utils.run_bass_kernel_spmd(
        nc, in_maps, core_ids=list(range(NCORES)), trace=TRACE)
    LAST_EXEC_TIME_NS = res.exec_time_ns
    LAST_RESULTS = res
    rows = np.concatenate([r['out'] for r in res.results], axis=0)
    return np.ascontiguousarray(rows[node_row].astype(np.float32))
